# revision 58
# baseline (speedup 1.0000x reference)
"""Multi-head attention (B=4, S=2048, D=1024, H=16) on 8 TRN2 NeuronCores.

Sharding: core c -> (batch b = c//2, head-group g = c%2): each core runs 8
heads of one batch (dout slice of 512) and emits two fp16 out-projection
partials (pairs 0-1 and 2-3); the host sums 4 partials per batch + bias.

All matmul operands are bf16 (fp32 PSUM accumulation); exp runs on the Act
engine (fp32 psum -> bf16). The AV product runs OPERAND-SWAPPED: the
[128k x 128q] exp tile is the PE *stationary* operand and the [128k x 65]
v_aug slice (64 v-dims + ones column for the softmax row-sum) is the
*moving* operand, so each accumulation step streams only 65 columns
(65-cycle matmul) instead of 512 — AV drops from 262k to 133k PE cycles.
The resulting attention output lands as [query, dim] in PSUM; softmax
normalization is a DVE reciprocal of the row-sum column plus a
per-partition-scalar multiply, and the [q, d] -> [d, q] layout flip for
the out-projection is done by XBAR DMA transposes (16x128 tiles, off-PE).
v-projection is computed directly in transposed [seq, dout] layout. The
v bias is folded into the host-side output bias (softmax rows sum to 1).

Schedule: k-proj (pairs 0-2) + q-proj(pair0, chunk0) prologue with
interleaved DMA sequencing; 16 attention blocks (pair a, 512-wide query
chunk j) run a depth-2 software pipeline over key-tile PAIRS (step p emits
fillers, QK/exp(p+2), AV(p)); scores for (2p, 2p+1) share one [128,2,512]
psum tile per head so exp stays 1024-wide. PSUM: score ring 2x[128,2,512]
(4 banks) + AV accumulators 2x[128,4,65] (2 banks, one per head per
block) + a DEDICATED filler pool 2x[128,512] (2 banks) — filler matmuls
(vT-proj, q/k-proj, out-proj) never touch the score ring. The last two QK
slots of each block prefetch the next block; out-proj groups run as tail
fillers and a short epilogue. Out-proj PSUM->SBUF copies alternate
DVE/Pool so the Act engine runs exp only. Out-projection partials are
written as fp16; the pair-2/3 tail and epilogue stores stage into
[128,8,512] tiles shipped as 2-dm chunked DMAs. PE ~274 us busy and
Act/exp ~266 us are co-critical; DVE ~105 us, Pool ~55 us, DMA ~100 us
hide under them.
"""
from contextlib import ExitStack

import ml_dtypes
import numpy as np

import concourse.bacc as bacc
import concourse.tile as tile
from concourse import mybir
from concourse.bass_utils import run_bass_kernel_spmd

F32 = mybir.dt.float32
F16 = mybir.dt.float16
ALU_ADD = mybir.AluOpType.add
BF = mybir.dt.bfloat16
AF = mybir.ActivationFunctionType
NPBF = ml_dtypes.bfloat16

B, S, D, H, HD = 4, 2048, 1024, 16, 64
GS = D // 2            # 512: per-core dout slice (8 heads, 4 pairs)
NP = GS // 128         # 4 head pairs (= dout tiles = wo k-tiles)
NK = D // 128          # 8 din k-tiles
NSK = S // 128         # 16 key tiles
SQ = 512               # query chunk (block width)
NSQ = S // SQ          # 4
NIP = NSK // 2         # 8 key-tile pairs per block
NCH = S // 512         # 4 (512-wide chunks of S)

_CACHE = {}


def _build_nc():
    if "nc" in _CACHE:
        return _CACHE["nc"]

    nc = bacc.Bacc()

    xqT = nc.dram_tensor("xqT", [128, NK, S], BF, kind="ExternalInput")
    xkT = nc.dram_tensor("xkT", [128, NK, S], BF, kind="ExternalInput")
    xvT = nc.dram_tensor("xvT", [128, NK, S], BF, kind="ExternalInput")
    wqT = nc.dram_tensor("wqT", [128, NK, GS], BF, kind="ExternalInput")
    wkT = nc.dram_tensor("wkT", [128, NK, GS], BF, kind="ExternalInput")
    wvT = nc.dram_tensor("wvT", [128, NK, GS], BF, kind="ExternalInput")
    woT = nc.dram_tensor("woT", [128, NP, D], BF, kind="ExternalInput")
    identT = nc.dram_tensor("identT", [128, 128], BF, kind="ExternalInput")
    biasqk = nc.dram_tensor("biasqk", [128, 8], F32, kind="ExternalInput")
    outTs = [nc.dram_tensor(f"outT{p}", [128, NK, S], F16,
                            kind="ExternalOutput") for p in range(2)]

    with tile.TileContext(nc) as tc, ExitStack() as kctx:
        consts = kctx.enter_context(tc.tile_pool(name="consts", bufs=1))
        pool_w = kctx.enter_context(tc.tile_pool(name="wp", bufs=1))
        pool_xq = kctx.enter_context(tc.tile_pool(name="xqp", bufs=1))
        pool_xs = kctx.enter_context(tc.tile_pool(name="xsp", bufs=3))
        pool_k = kctx.enter_context(tc.tile_pool(name="kTp", bufs=2))
        pool_q = kctx.enter_context(tc.tile_pool(name="qTp", bufs=2))
        pool_va = kctx.enter_context(tc.tile_pool(name="vap", bufs=1))
        pool_e = kctx.enter_context(tc.tile_pool(name="ep", bufs=16))
        pool_oT = kctx.enter_context(tc.tile_pool(name="oTp", bufs=1))
        pool_rr = kctx.enter_context(tc.tile_pool(name="rrp", bufs=3))
        pool_ps = kctx.enter_context(tc.tile_pool(name="psb", bufs=2))
        pool_oq = kctx.enter_context(tc.tile_pool(name="oqp", bufs=2))
        pool_oo = kctx.enter_context(tc.tile_pool(name="oop", bufs=5))
        pool_ob = kctx.enter_context(tc.tile_pool(name="obp", bufs=2))
        pp_qk = kctx.enter_context(tc.tile_pool(name="ppqk", bufs=2,
                                                space="PSUM"))
        pp_av = kctx.enter_context(tc.tile_pool(name="ppav", bufs=2,
                                                space="PSUM"))
        pp_fill = kctx.enter_context(tc.tile_pool(name="ppfl", bufs=2,
                                                  space="PSUM"))

        bias_t = consts.tile([128, 8], F32)

        # ---------------- static SBUF tensors ----------------
        wk_t = pool_w.tile([128, NK, GS], BF, name="wk")
        wq_t = pool_w.tile([128, NK, GS], BF, name="wq")
        wv_t = pool_w.tile([128, NK, GS], BF, name="wv")
        wo_t = pool_w.tile([128, NP, D], BF, name="wo")
        ident_t = pool_w.tile([128, 128], BF, name="ident")
        xq_t = pool_xq.tile([128, NK, S], BF, name="xq")
        kT = [pool_k.tile([128, S], BF, name=f"kT{m}") for m in range(NP)]
        v_aug = [pool_va.tile([128, 8, HD + 1], BF, name=f"va{i}")
                 for i in range(NSK)]
        # one tile per (pair, 512-query chunk): keeps the out-proj read
        # dependency scoped to the chunk it actually consumes (a single
        # [128, S] tile would couple every out-proj filler to the LATEST
        # norm transposes via whole-tile DMA-write deps)
        o_tiles = [[pool_oT.tile([128, SQ], BF, name=f"oT{a}{j}")
                    for j in range(NCH)] for a in range(NP)]
        q_tiles = {}

        # ---------------- prologue: k-proj (all pairs) ----------------
        nc.sync.dma_start(out=wk_t[:, 0:4, 0:256], in_=wkT[:, 0:4, 0:256])
        xk_tiles = {}

        def xk_dma(n, split=False):
            xk_tiles[n] = pool_xs.tile([128, NK, 512], BF, tag="xs",
                                       name=f"xk{n}")
            if split:
                for q in range(4):
                    nc.sync.dma_start(
                        out=xk_tiles[n][:, 2 * q:2 * q + 2, :],
                        in_=xkT[:, 2 * q:2 * q + 2,
                                n * 512:(n + 1) * 512])
            else:
                nc.sync.dma_start(out=xk_tiles[n],
                                  in_=xkT[:, :, n * 512:(n + 1) * 512])

        def kproj_group(m, n, tiles):
            ps = pp_fill.tile([128, 512], F32, tag="fl", name=f"psk{n}{m}")
            for kk in range(NK):
                nc.tensor.matmul(
                    ps[:],
                    wk_t[:, kk, m * 128:(m + 1) * 128],
                    tiles[n][:, kk, :],
                    start=(kk == 0),
                    stop=(kk == NK - 1),
                )
            nc.vector.tensor_scalar_add(
                kT[m][:, n * 512:(n + 1) * 512], ps[:],
                bias_t[:, 4 + m:5 + m])

        def kproj_fillers(m):
            """4 filler groups for k-proj of pair m (re-streams xk)."""
            tiles = {}

            def dma(n):
                tiles[n] = pool_xs.tile([128, NK, 512], BF, tag="xs",
                                        name=f"xk{m}_{n}")
                nc.sync.dma_start(out=tiles[n],
                                  in_=xkT[:, :, n * 512:(n + 1) * 512])

            def group(n):
                def run():
                    if n + 1 < NCH:
                        dma(n + 1)
                    kproj_group(m, n, tiles)
                return run
            return [group(n) for n in range(NCH)], dma

        def xq_dma(n):
            nc.sync.dma_start(out=xq_t[:, :, n * 512:(n + 1) * 512],
                              in_=xqT[:, :, n * 512:(n + 1) * 512])

        def qproj_groups(a):
            qt = pool_q.tile([128, S], BF, tag="qT", name=f"qT{a}")
            q_tiles[a] = qt

            def group(n):
                def run():
                    ps = pp_fill.tile([128, 512], F32, tag="fl",
                                     name=f"psq{a}{n}")
                    for kk in range(NK):
                        nc.tensor.matmul(
                            ps[:],
                            wq_t[:, kk, a * 128:(a + 1) * 128],
                            xq_t[:, kk, n * 512:(n + 1) * 512],
                            start=(kk == 0),
                            stop=(kk == NK - 1),
                        )
                    nc.vector.tensor_scalar_add(
                        qt[:, n * 512:(n + 1) * 512], ps[:],
                        bias_t[:, a:a + 1])
                return run
            return [group(n) for n in range(NCH)]

        q0 = qproj_groups(0)

        # ---------------- vT-proj groups (one per seq-tile st) ----------
        xv_tiles = {}

        def xv_dma(n):
            xv_tiles[n] = pool_xs.tile([128, NK, 512], BF, tag="xs",
                                       name=f"xv{n}")
            nc.sync.dma_start(out=xv_tiles[n],
                              in_=xvT[:, :, n * 512:(n + 1) * 512])

        def vt_group(st):
            def run():
                n, sl = st // 4, st % 4
                if sl == 0 and 1 <= n < NCH - 1:
                    xv_dma(n + 1)
                ps = pp_fill.tile([128, 512], F32, tag="fl", name=f"psv{st}")
                for kk in range(NK):
                    nc.tensor.matmul(
                        ps[:],
                        xv_tiles[n][:, kk, sl * 128:(sl + 1) * 128],
                        wv_t[:, kk, :],
                        start=(kk == 0),
                        stop=(kk == NK - 1),
                    )
                nc.vector.memset(v_aug[st][:, :, HD:HD + 1], 1.0)
                nc.vector.tensor_copy(v_aug[st][:, :, 0:HD], ps[:])
            return run

        vt_fill = [vt_group(st) for st in range(NSK)]

        # ---------------- out-proj groups ----------------
        def outproj_groups(p, jjs, copy_eng="vector", pool_alt=False,
                           pairs=None):
            plist = pairs if pairs is not None else (2 * p, 2 * p + 1)

            def group(dm, jj, gi):
                def run():
                    if pool_alt and gi % 2:
                        # post-exp groups: the score ring is idle, borrow its
                        # slots to double the effective psum ring depth
                        ps = pp_qk.tile([128, 512], F32, tag="sc",
                                        name=f"pso{p}{dm}{jj}")
                    else:
                        ps = pp_fill.tile([128, 512], F32, tag="fl",
                                         name=f"pso{p}{dm}{jj}")
                    for a in plist:
                        nc.tensor.matmul(
                            ps[:],
                            wo_t[:, a, dm * 128:(dm + 1) * 128],
                            o_tiles[a][jj][:, :],
                            start=(a == plist[0]),
                            stop=(a == plist[-1]),
                        )
                    oo = pool_oo.tile([128, 512], F16, tag="oo",
                                      name=f"oo{p}{dm}{jj}")
                    use_act = (copy_eng == "scalar"
                               or (copy_eng == "alt" and gi % 2))
                    with nc.allow_low_precision(reason="fp16 partial out"):
                        if use_act:
                            nc.scalar.copy(oo[:], ps[:])
                        else:
                            nc.vector.tensor_copy(oo[:], ps[:])
                    nc.sync.dma_start(
                        out=outTs[p][:, dm, jj * 512:(jj + 1) * 512],
                        in_=oo[:])
                return run
            return [group(dm, jj, gi)
                    for gi, (jj, dm) in enumerate(
                        (jj, dm) for jj in jjs for dm in range(NK))]

        # ---------------- attention ----------------
        def make_qk_exp(a, j, ee):
            def qk_exp(p):
                for h in range(2):
                    hb = h * HD
                    sc = pp_qk.tile([128, 2, 512], F32, tag="sc",
                                    name=f"sc{a}{j}{p}{h}")
                    for u in range(2):
                        i = 2 * p + u
                        nc.tensor.matmul(
                            sc[:, u, :],
                            kT[a][hb:hb + HD, i * 128:(i + 1) * 128],
                            q_tiles[a][hb:hb + HD,
                                       j * SQ:(j + 1) * SQ],
                            start=True,
                            stop=True,
                        )
                    e = pool_e.tile([128, 2, 512], BF, tag="e",
                                    name=f"e{a}{j}{p}{h}")
                    nc.scalar.activation(e[:], sc[:], AF.Exp)
                    ee[(p, h)] = e
            return qk_exp

        def av_step(po_, ee_, a_, p):
            """AV for key-tile pair p, both heads, operand-swapped: exp tile
            slices [128k x 128q] are stationary, v_aug [128k x 65] moves —
            65-cycle accumulation steps into [128q, NP, 65] accumulators.
            ONE accumulation group per po bank: start=True zeroes the whole
            PSUM bank, so only the very first matmul of the block carries
            it; every other (qt, i) accumulates into regions the initial
            wipe zeroed."""
            for h in range(2):
                e = ee_.pop((p, h))
                for qt in range(NP):
                    for u in range(2):
                        nc.tensor.matmul(
                            po_[h][:, qt, :],
                            e[:, u, qt * 128:(qt + 1) * 128],
                            v_aug[2 * p + u][:, 2 * a_ + h, :],
                            start=(p == 0 and qt == 0 and u == 0),
                            stop=(p == NIP - 1 and qt == NP - 1
                                  and u == 1),
                            skip_group_check=True,
                        )

        def norm_begin(po_, a_, j_):
            """Reciprocals of the softmax row-sum columns + oq staging tile."""
            with nc.allow_low_precision(reason="bf16 softmax reciprocal"):
                rrs = []
                for h in range(2):
                    rr = pool_rr.tile([128, NP], F32, tag="rr",
                                      name=f"rr{a_}{j_}{h}")
                    nc.vector.reciprocal(rr[:], po_[h][:, :, HD:HD + 1])
                    rrs.append(rr)
                oq = pool_oq.tile([128, NP, 128], BF, tag="oq",
                                  name=f"oq{a_}{j_}")
            return (po_, a_, j_, rrs, oq)

        def norm_part(ns, qts, pe_transpose=False, copy_eng="vector"):
            """Per-partition-scalar normalization multiplies for the given
            query tiles, then [q, d] -> [d, q] via XBAR DMA transposes (or
            PE transposes + DVE copies on the drain path where DMA latency
            would be exposed). qt must run DESCENDING overall: a later
            block's bank-wiping start=True is dep-tracked only against
            region qt0, so qt0's reads must be the LAST norm ops (DVE is
            in-order) for the wipe to serialize behind every read."""
            po_, a_, j_, rrs, oq = ns
            with nc.allow_low_precision(reason="bf16 softmax normalize"):
                for qt in qts:
                    dst = o_tiles[a_][j_][:, qt * 128:(qt + 1) * 128]
                    for h in range(2):
                        nc.vector.tensor_scalar_mul(
                            oq[:, qt, h * HD:(h + 1) * HD],
                            po_[h][:, qt, 0:HD],
                            rrs[h][:, qt:qt + 1])
                    if pe_transpose:
                        pt = pp_fill.tile([128, 128], BF, tag="fl",
                                          name=f"pt{a_}{j_}{qt}")
                        nc.tensor.transpose(pt[:], oq[:, qt, :], ident_t[:])
                        if copy_eng == "scalar":
                            nc.scalar.copy(dst, pt[:])
                        else:
                            nc.vector.tensor_copy(dst, pt[:])
                    else:
                        nc.sync.dma_start_transpose(out=dst,
                                                    in_=oq[:, qt, :])

        def attn_block(a, j, fillers, pre, nxt, carry, pace_off=1,
                       tail_fillers=(), pre_emitted=3):
            """Depth-5 software pipeline over key-tile PAIRS: step p emits
            [QK/exp(p+3), AV(p-2), fillers]; the block's last two AV steps
            and its normalization run in the NEXT block (steps 0-2), so the
            po bank-reuse distance is a full block and the bank-wiping
            start of each AV group never stalls on norm reads. QK/exp runs
            3 steps ahead of issue so the Act exp queue stays fed across
            block-boundary filler stalls; fillers are emitted LAST in each
            step so a stalled filler (psum-ring WAR on a DVE copy) never
            gates the QK convoy. Scores for (2p, 2p+1) share one
            [128,2,512] psum tile per head so exp stays 1024-wide; filler
            matmuls use their own pp_fill pool."""
            po = [pp_av.tile([128, NP, HD + 1], F32, tag="po",
                             name=f"po{a}{j}{h}") for h in range(2)]
            ee = pre
            qk_exp = make_qk_exp(a, j, ee)
            nee = {}
            nqk = make_qk_exp(nxt[0], nxt[1], nee) if nxt else None
            nf, fi = len(fillers), 0
            c_ns = None
            for p in range(NIP):
                if p + 3 < NIP:
                    if p + 3 >= pre_emitted:
                        qk_exp(p + 3)
                elif nqk is not None:
                    nqk(p + 3 - NIP)
                if p < 2:
                    if carry is not None and p == 0:
                        c_po, c_ee, c_a, c_j = carry
                        av_step(c_po, c_ee, c_a, NIP - 2)
                        av_step(c_po, c_ee, c_a, NIP - 1)
                else:
                    av_step(po, ee, a, p - 2)
                if carry is not None and p == 1:
                    # bulk-copy the carry po accumulators to SBUF: ONE fast
                    # DVE read per head frees the po PSUM slots immediately,
                    # so this block's own bank-wiping AV start (p=2) never
                    # waits on the spread-out normalization reads
                    c_po, c_ee, c_a, c_j = carry
                    c_sb = []
                    for h in range(2):
                        sb = pool_ps.tile([128, NP, HD + 1], F32, tag="ps",
                                          name=f"psb{c_a}{c_j}{h}")
                        nc.vector.tensor_copy(sb[:], c_po[h][:])
                        c_sb.append(sb)
                    c_ns = norm_begin(c_sb, c_a, c_j)
                if carry is not None and 2 <= p <= 5:
                    # norm spread ONE query-tile per step: the DVE stream
                    # stays shallow so filler psum-ring copies behind it
                    # release promptly (a norm burst convoys PE via the
                    # in-order DVE queue)
                    norm_part(c_ns, (5 - p,), pe_transpose=True)
                want = -(-(p + pace_off) * nf // NIP)
                while fi < min(want, nf):
                    fillers[fi]()
                    fi += 1
            while fi < nf:
                fillers[fi]()
                fi += 1
            for g in tail_fillers:
                g()
            return nee, (po, ee, a, j)

        q1 = qproj_groups(1)
        q2 = qproj_groups(2)
        q3 = qproj_groups(3)
        k1, k1_dma = kproj_fillers(1)
        k2, k2_dma = kproj_fillers(2)
        k3, k3_dma = kproj_fillers(3)
        op01 = outproj_groups(0, (0, 1, 2, 3))
        op23_0 = outproj_groups(1, (0,))
        op23_1 = outproj_groups(1, (1,))
        def merged_op23(jj, pairs=(2, 3), all_sc=False, ship_each=False):
            """out-proj pairs 2-3 for query chunk jj, staged into 2-dm
            chunk tiles shipped as single DMAs (one HWDGE fixed cost per
            chunk, transfers overlapping the remaining units). ship_each
            ships 1-dm right after each copy — shorter drain for the
            epilogue at the cost of more HWDGE issues."""
            def unit(dm, obc):
                def run():
                    if dm % 2 or all_sc:
                        ps = pp_qk.tile([128, 512], F32, tag="sc",
                                        name=f"pst{jj}{dm}")
                    else:
                        ps = pp_fill.tile([128, 512], F32, tag="fl",
                                         name=f"pst{jj}{dm}")
                    for a in pairs:
                        nc.tensor.matmul(
                            ps[:],
                            wo_t[:, a, dm * 128:(dm + 1) * 128],
                            o_tiles[a][jj][:, :],
                            start=(a == pairs[0]),
                            stop=(a == pairs[-1]),
                        )
                    with nc.allow_low_precision(reason="fp16 partial out"):
                        if dm % 2:
                            nc.scalar.copy(obc[:, dm % 2, :], ps[:])
                        else:
                            nc.vector.tensor_copy(obc[:, dm % 2, :], ps[:])
                return run

            def ship(obc, d0, nd, sl):
                def run():
                    nc.sync.dma_start(
                        out=outTs[1][:, d0:d0 + nd, jj * 512:(jj + 1) * 512],
                        in_=obc[:, sl:sl + nd, :])
                return run

            seq = []
            obc = None
            for dm in range(NK):
                if dm % 2 == 0:
                    obc = pool_ob.tile([128, 2, 512], F16, tag="ob",
                                       name=f"ob{jj}{dm}")
                seq.append(unit(dm, obc))
                if ship_each:
                    seq.append(ship(obc, dm, 1, dm % 2))
                elif dm % 2:
                    seq.append(ship(obc, dm - 1, 2, 0))
            return seq

        op23_2t = merged_op23(2)

        # ---------------- prologue ----------------
        # DMA issue order == consumer order (the modeled DMA transfer device
        # is serial): wk/xk0 then wq/xq0 so the FIRST QK/exp fires ~14 us in;
        # the Act exp chain is the serial long pole, so everything upstream
        # of it is prioritized. k-proj pairs 1-3 re-stream xk later as
        # mid-schedule fillers inside Act-bound blocks.
        xk_dma(0, split=True)
        nc.sync.dma_start(out=wk_t[:, 4:8, 0:256], in_=wkT[:, 4:8, 0:256])
        nc.sync.dma_start(out=bias_t, in_=biasqk[:, :])
        nc.sync.dma_start(out=wq_t, in_=wqT[:, :, :])
        xq_dma(0)
        xk_dma(1)
        kproj_group(0, 0, xk_tiles)
        q0[0]()
        ee0 = {}
        qk00 = make_qk_exp(0, 0, ee0)
        qk00(0)
        qk00(1)
        xk_dma(2)
        kproj_group(0, 1, xk_tiles)
        qk00(2)
        xk_dma(3)
        nc.sync.dma_start(out=wv_t, in_=wvT[:, :, :])
        xv_dma(0)
        kproj_group(0, 2, xk_tiles)
        xv_dma(1)
        kproj_group(0, 3, xk_tiles)
        vt_fill[0]()
        vt_fill[1]()
        xq_dma(1)

        # ------- block (0,0): custom DMA-arrival-aware schedule ----------
        # QK/exp calls pace the Act chain; vt-proj fillers (gated by the
        # serial xv stream) slot between them; AV lags far enough that its
        # v_aug inputs exist. (0,1)'s first five QK calls are pre-emitted
        # here so Act stays fed through the v-proj PE hump at the boundary.
        po00 = [pp_av.tile([128, NP, HD + 1], F32, tag="po",
                           name=f"po00{h}") for h in range(2)]
        nee01 = {}
        qk01 = make_qk_exp(0, 1, nee01)
        qk00(3)
        qk00(4)
        vt_fill[2]()
        vt_fill[3]()
        qk00(5)
        q0[1]()
        qk00(6)
        vt_fill[4]()
        vt_fill[5]()
        qk00(7)
        av_step(po00, ee0, 0, 0)
        qk01(0)
        vt_fill[6]()
        vt_fill[7]()
        av_step(po00, ee0, 0, 1)
        qk01(1)
        vt_fill[8]()
        vt_fill[9]()
        av_step(po00, ee0, 0, 2)
        qk01(2)
        vt_fill[10]()
        vt_fill[11]()
        av_step(po00, ee0, 0, 3)
        qk01(3)
        vt_fill[12]()
        vt_fill[13]()
        av_step(po00, ee0, 0, 4)
        qk01(4)
        av_step(po00, ee0, 0, 5)
        vt_fill[14]()
        vt_fill[15]()
        qk01(5)
        # late-consumer DMAs issued after the xv stream they must not delay
        xq_dma(2)
        xq_dma(3)
        nc.sync.dma_start(out=wk_t[:, :, 256:512], in_=wkT[:, :, 256:512])
        nc.sync.dma_start(out=wo_t, in_=woT[:, :, :])
        nc.sync.dma_start(out=ident_t, in_=identT[:, :])
        k1_dma(0)

        plan = [
            (0, 1, [q0[2], q0[3]], 1, (), 6),
            (0, 2, [k1[0], k1[1], q1[0]], 1, (), 3),
            (0, 3, [k1[2], k1[3], q1[1], lambda: k2_dma(0)], 1, (), 3),
            (1, 0, [q1[2], k2[0], q1[3]], 2, (), 3),
            (1, 1, [k2[1], q2[0]] + op01[0:1], 2, (), 3),
            (1, 2, [k2[2], q2[1]] + op01[1:3], 2, (), 3),
            (1, 3, [k2[3], q2[2]] + op01[3:5] + [lambda: k3_dma(0)],
             2, (), 3),
            (2, 0, [q2[3], k3[0]] + op01[5:8], 2, (), 3),
            (2, 1, [q3[0], k3[1]] + op01[8:11], 2, (), 3),
            (2, 2, [q3[1], k3[2]] + op01[11:14], 2, (), 3),
            (2, 3, [k3[3], q3[2]] + op01[14:17], 2, (), 3),
            (3, 0, [q3[3]] + op01[17:24], 2, (), 3),
            (3, 1, op01[24:32], 2, (), 3),
            (3, 2, op23_0, 2, (), 3),
            # tail fillers (alt copies) hide the final norm chain
            (3, 3, op23_1, 2, op23_2t, 3),
        ]
        pre, carry = nee01, (po00, ee0, 0, 0)
        for bi, (a, j, fillers, off, tails, pe_d) in enumerate(plan):
            nxt = plan[bi + 1][0:2] if bi + 1 < len(plan) else None
            pre, carry = attn_block(a, j, fillers, pre, nxt, carry,
                                    pace_off=off, tail_fillers=tails,
                                    pre_emitted=pe_d)
        # flush: final block's last two AV steps + norm (PE transposes so
        # the drain doesn't pay DMA-transpose issue+sem latency)
        c_po, c_ee, c_a, c_j = carry
        av_step(c_po, c_ee, c_a, NIP - 2)
        av_step(c_po, c_ee, c_a, NIP - 1)
        ns = norm_begin(c_po, c_a, c_j)
        norm_part(ns, (3, 2, 1, 0), pe_transpose=True)
        # epilogue (out-proj pairs 2-3, jj=3): all-sc psum (the score ring
        # is idle at the drain; the fl ring serves the PE transposes)
        for g in merged_op23(3, ship_each=True):
            g()

    nc.compile()
    _CACHE["nc"] = nc
    return nc


def _tox(a):
    """[1024|512, N] -> [128, k, N] bf16 (partition-major k-tiling)."""
    r = a.shape[0] // 128
    return np.ascontiguousarray(
        a.reshape(r, 128, a.shape[1]).transpose(1, 0, 2)).astype(NPBF)


def kernel(Q, K, V, Wq, bq, Wk, bk, Wv, bv, Wo, bo):
    Q = np.asarray(Q, np.float32)
    K = np.asarray(K, np.float32)
    V = np.asarray(V, np.float32)
    Wq = np.asarray(Wq, np.float32)
    Wk = np.asarray(Wk, np.float32)
    Wv = np.asarray(Wv, np.float32)
    Wo = np.asarray(Wo, np.float32)
    bq = np.asarray(bq, np.float32)
    bk = np.asarray(bk, np.float32)
    bv = np.asarray(bv, np.float32)
    bo = np.asarray(bo, np.float32)
    scale = 1.0 / 8.0  # 1/sqrt(HD), folded into the q projection

    nc = _build_nc()
    in_maps = []
    for c in range(8):
        b, g = divmod(c, 2)
        gs = slice(g * GS, (g + 1) * GS)
        biasqk = np.empty((128, 8), np.float32)
        for m in range(NP):
            biasqk[:, m] = bq[gs][m * 128:(m + 1) * 128] * scale
            biasqk[:, 4 + m] = bk[gs][m * 128:(m + 1) * 128]
        in_maps.append({
            "xqT": _tox(Q[b].T),
            "xkT": _tox(K[b].T),
            "xvT": _tox(V[b].T),
            "wqT": _tox((Wq[gs] * scale).T),
            "wkT": _tox(Wk[gs].T),
            "wvT": _tox(Wv[gs].T),
            "woT": _tox(Wo[:, gs].T),
            "identT": np.eye(128, dtype=NPBF),
            "biasqk": biasqk,
        })

    host_bias = bo + Wo @ bv  # v bias folded through softmax + out-proj

    def run_and_gather():
        res = run_bass_kernel_spmd(nc, in_maps, list(range(8)))
        out = np.empty((B, S, D), np.float32)
        for b in range(B):
            acc = None
            for c in (2 * b, 2 * b + 1):
                for p in range(2):
                    part = np.asarray(res.results[c][f"outT{p}"])
                    part = part.transpose(1, 0, 2).reshape(D, S)
                    acc = part if acc is None else acc + part
            out[b] = acc.T + host_bias
        return out

    try:
        return run_and_gather()
    except Exception:
        # transient device wedge (e.g. NRT_EXEC_UNIT_UNRECOVERABLE) can
        # surface either in the run or in result materialization: retry once
        return run_and_gather()



# revision 59
# speedup vs baseline: 1.0035x; 1.0035x over previous
"""Multi-head attention (B=4, S=2048, D=1024, H=16) on 8 TRN2 NeuronCores.

Sharding: core c -> (batch b = c//2, head-group g = c%2): each core runs 8
heads of one batch (dout slice of 512) and emits two fp16 out-projection
partials (pairs 0-1 and 2-3); the host sums 4 partials per batch + bias.

All matmul operands are bf16 (fp32 PSUM accumulation); exp runs on the Act
engine (fp32 psum -> bf16). The AV product runs OPERAND-SWAPPED: the
[128k x 128q] exp tile is the PE *stationary* operand and the [128k x 65]
v_aug slice (64 v-dims + ones column for the softmax row-sum) is the
*moving* operand, so each accumulation step streams only 65 columns
(65-cycle matmul) instead of 512 — AV drops from 262k to 133k PE cycles.
The resulting attention output lands as [query, dim] in PSUM; softmax
normalization is a DVE reciprocal of the row-sum column plus a
per-partition-scalar multiply, and the [q, d] -> [d, q] layout flip for
the out-projection is done by XBAR DMA transposes (16x128 tiles, off-PE).
v-projection is computed directly in transposed [seq, dout] layout. The
v bias is folded into the host-side output bias (softmax rows sum to 1).

Schedule: k-proj (pairs 0-2) + q-proj(pair0, chunk0) prologue with
interleaved DMA sequencing; 16 attention blocks (pair a, 512-wide query
chunk j) run a depth-2 software pipeline over key-tile PAIRS (step p emits
fillers, QK/exp(p+2), AV(p)); scores for (2p, 2p+1) share one [128,2,512]
psum tile per head so exp stays 1024-wide. PSUM: score ring 2x[128,2,512]
(4 banks) + AV accumulators 2x[128,4,65] (2 banks, one per head per
block) + a DEDICATED filler pool 2x[128,512] (2 banks) — filler matmuls
(vT-proj, q/k-proj, out-proj) never touch the score ring. The last two QK
slots of each block prefetch the next block; out-proj groups run as tail
fillers and a short epilogue. Out-proj PSUM->SBUF copies alternate
DVE/Pool so the Act engine runs exp only. Out-projection partials are
written as fp16; the pair-2/3 tail and epilogue stores stage into
[128,8,512] tiles shipped as 2-dm chunked DMAs. PE ~274 us busy and
Act/exp ~266 us are co-critical; DVE ~105 us, Pool ~55 us, DMA ~100 us
hide under them.
"""
from contextlib import ExitStack

import ml_dtypes
import numpy as np

import concourse.bacc as bacc
import concourse.tile as tile
from concourse import mybir
from concourse.bass_utils import run_bass_kernel_spmd

F32 = mybir.dt.float32
F16 = mybir.dt.float16
ALU_ADD = mybir.AluOpType.add
BF = mybir.dt.bfloat16
AF = mybir.ActivationFunctionType
NPBF = ml_dtypes.bfloat16

B, S, D, H, HD = 4, 2048, 1024, 16, 64
GS = D // 2            # 512: per-core dout slice (8 heads, 4 pairs)
NP = GS // 128         # 4 head pairs (= dout tiles = wo k-tiles)
NK = D // 128          # 8 din k-tiles
NSK = S // 128         # 16 key tiles
SQ = 512               # query chunk (block width)
NSQ = S // SQ          # 4
NIP = NSK // 2         # 8 key-tile pairs per block
NCH = S // 512         # 4 (512-wide chunks of S)

_CACHE = {}


def _build_nc():
    if "nc" in _CACHE:
        return _CACHE["nc"]

    nc = bacc.Bacc()

    xqT = nc.dram_tensor("xqT", [128, NK, S], BF, kind="ExternalInput")
    xkT = nc.dram_tensor("xkT", [128, NK, S], BF, kind="ExternalInput")
    xvT = nc.dram_tensor("xvT", [128, NK, S], BF, kind="ExternalInput")
    wqT = nc.dram_tensor("wqT", [128, NK, GS], BF, kind="ExternalInput")
    wkT = nc.dram_tensor("wkT", [128, NK, GS], BF, kind="ExternalInput")
    wvT = nc.dram_tensor("wvT", [128, NK, GS], BF, kind="ExternalInput")
    woT = nc.dram_tensor("woT", [128, NP, D], BF, kind="ExternalInput")
    identT = nc.dram_tensor("identT", [128, 128], BF, kind="ExternalInput")
    biasqk = nc.dram_tensor("biasqk", [128, 8], F32, kind="ExternalInput")
    outTs = [nc.dram_tensor(f"outT{p}", [128, NK, S], F16,
                            kind="ExternalOutput") for p in range(2)]

    with tile.TileContext(nc) as tc, ExitStack() as kctx:
        consts = kctx.enter_context(tc.tile_pool(name="consts", bufs=1))
        pool_w = kctx.enter_context(tc.tile_pool(name="wp", bufs=1))
        pool_xq = kctx.enter_context(tc.tile_pool(name="xqp", bufs=1))
        pool_xs = kctx.enter_context(tc.tile_pool(name="xsp", bufs=3))
        pool_k = kctx.enter_context(tc.tile_pool(name="kTp", bufs=2))
        pool_q = kctx.enter_context(tc.tile_pool(name="qTp", bufs=2))
        pool_va = kctx.enter_context(tc.tile_pool(name="vap", bufs=1))
        pool_e = kctx.enter_context(tc.tile_pool(name="ep", bufs=16))
        pool_oT = kctx.enter_context(tc.tile_pool(name="oTp", bufs=1))
        pool_rr = kctx.enter_context(tc.tile_pool(name="rrp", bufs=3))
        pool_ps = kctx.enter_context(tc.tile_pool(name="psb", bufs=2))
        pool_oq = kctx.enter_context(tc.tile_pool(name="oqp", bufs=2))
        pool_oo = kctx.enter_context(tc.tile_pool(name="oop", bufs=5))
        pool_ob = kctx.enter_context(tc.tile_pool(name="obp", bufs=2))
        pp_qk = kctx.enter_context(tc.tile_pool(name="ppqk", bufs=2,
                                                space="PSUM"))
        pp_av = kctx.enter_context(tc.tile_pool(name="ppav", bufs=2,
                                                space="PSUM"))
        pp_fill = kctx.enter_context(tc.tile_pool(name="ppfl", bufs=2,
                                                  space="PSUM"))

        bias_t = consts.tile([128, 8], F32)

        # ---------------- static SBUF tensors ----------------
        wk_t = pool_w.tile([128, NK, GS], BF, name="wk")
        wq_t = pool_w.tile([128, NK, GS], BF, name="wq")
        wv_t = pool_w.tile([128, NK, GS], BF, name="wv")
        wo_t = pool_w.tile([128, NP, D], BF, name="wo")
        ident_t = pool_w.tile([128, 128], BF, name="ident")
        xq_t = pool_xq.tile([128, NK, S], BF, name="xq")
        kT = [pool_k.tile([128, S], BF, name=f"kT{m}") for m in range(NP)]
        v_aug = [pool_va.tile([128, 8, HD + 1], BF, name=f"va{i}")
                 for i in range(NSK)]
        # one tile per (pair, 512-query chunk): keeps the out-proj read
        # dependency scoped to the chunk it actually consumes (a single
        # [128, S] tile would couple every out-proj filler to the LATEST
        # norm transposes via whole-tile DMA-write deps)
        o_tiles = [[pool_oT.tile([128, SQ], BF, name=f"oT{a}{j}")
                    for j in range(NCH)] for a in range(NP)]
        q_tiles = {}

        # ---------------- prologue: k-proj (all pairs) ----------------
        nc.sync.dma_start(out=wk_t[:, 0:4, 0:256], in_=wkT[:, 0:4, 0:256])
        xk_tiles = {}

        def xk_dma(n, split=False):
            xk_tiles[n] = pool_xs.tile([128, NK, 512], BF, tag="xs",
                                       name=f"xk{n}")
            if split:
                for q in range(4):
                    nc.sync.dma_start(
                        out=xk_tiles[n][:, 2 * q:2 * q + 2, :],
                        in_=xkT[:, 2 * q:2 * q + 2,
                                n * 512:(n + 1) * 512])
            else:
                nc.sync.dma_start(out=xk_tiles[n],
                                  in_=xkT[:, :, n * 512:(n + 1) * 512])

        def kproj_group(m, n, tiles):
            ps = pp_fill.tile([128, 512], F32, tag="fl", name=f"psk{n}{m}")
            for kk in range(NK):
                nc.tensor.matmul(
                    ps[:],
                    wk_t[:, kk, m * 128:(m + 1) * 128],
                    tiles[n][:, kk, :],
                    start=(kk == 0),
                    stop=(kk == NK - 1),
                )
            nc.vector.tensor_scalar_add(
                kT[m][:, n * 512:(n + 1) * 512], ps[:],
                bias_t[:, 4 + m:5 + m])

        def kproj_fillers(m):
            """4 filler groups for k-proj of pair m (re-streams xk)."""
            tiles = {}

            def dma(n):
                tiles[n] = pool_xs.tile([128, NK, 512], BF, tag="xs",
                                        name=f"xk{m}_{n}")
                nc.sync.dma_start(out=tiles[n],
                                  in_=xkT[:, :, n * 512:(n + 1) * 512])

            def group(n):
                def run():
                    if n + 1 < NCH:
                        dma(n + 1)
                    kproj_group(m, n, tiles)
                return run
            return [group(n) for n in range(NCH)], dma

        def xq_dma(n):
            nc.sync.dma_start(out=xq_t[:, :, n * 512:(n + 1) * 512],
                              in_=xqT[:, :, n * 512:(n + 1) * 512])

        def qproj_groups(a):
            qt = pool_q.tile([128, S], BF, tag="qT", name=f"qT{a}")
            q_tiles[a] = qt

            def group(n):
                def run():
                    ps = pp_fill.tile([128, 512], F32, tag="fl",
                                     name=f"psq{a}{n}")
                    for kk in range(NK):
                        nc.tensor.matmul(
                            ps[:],
                            wq_t[:, kk, a * 128:(a + 1) * 128],
                            xq_t[:, kk, n * 512:(n + 1) * 512],
                            start=(kk == 0),
                            stop=(kk == NK - 1),
                        )
                    nc.vector.tensor_scalar_add(
                        qt[:, n * 512:(n + 1) * 512], ps[:],
                        bias_t[:, a:a + 1])
                return run
            return [group(n) for n in range(NCH)]

        q0 = qproj_groups(0)

        # ---------------- vT-proj groups (one per seq-tile st) ----------
        xv_tiles = {}

        def xv_dma(n):
            xv_tiles[n] = pool_xs.tile([128, NK, 512], BF, tag="xs",
                                       name=f"xv{n}")
            nc.sync.dma_start(out=xv_tiles[n],
                              in_=xvT[:, :, n * 512:(n + 1) * 512])

        def vt_group(st):
            def run():
                n, sl = st // 4, st % 4
                if sl == 0 and 1 <= n < NCH - 1:
                    xv_dma(n + 1)
                ps = pp_fill.tile([128, 512], F32, tag="fl", name=f"psv{st}")
                for kk in range(NK):
                    nc.tensor.matmul(
                        ps[:],
                        xv_tiles[n][:, kk, sl * 128:(sl + 1) * 128],
                        wv_t[:, kk, :],
                        start=(kk == 0),
                        stop=(kk == NK - 1),
                    )
                nc.vector.memset(v_aug[st][:, :, HD:HD + 1], 1.0)
                nc.vector.tensor_copy(v_aug[st][:, :, 0:HD], ps[:])
            return run

        vt_fill = [vt_group(st) for st in range(NSK)]

        # ---------------- out-proj groups ----------------
        def outproj_groups(p, jjs, copy_eng="vector", pool_alt=False,
                           pairs=None):
            plist = pairs if pairs is not None else (2 * p, 2 * p + 1)

            def group(dm, jj, gi):
                def run():
                    if pool_alt and gi % 2:
                        # post-exp groups: the score ring is idle, borrow its
                        # slots to double the effective psum ring depth
                        ps = pp_qk.tile([128, 512], F32, tag="sc",
                                        name=f"pso{p}{dm}{jj}")
                    else:
                        ps = pp_fill.tile([128, 512], F32, tag="fl",
                                         name=f"pso{p}{dm}{jj}")
                    for a in plist:
                        nc.tensor.matmul(
                            ps[:],
                            wo_t[:, a, dm * 128:(dm + 1) * 128],
                            o_tiles[a][jj][:, :],
                            start=(a == plist[0]),
                            stop=(a == plist[-1]),
                        )
                    oo = pool_oo.tile([128, 512], F16, tag="oo",
                                      name=f"oo{p}{dm}{jj}")
                    use_act = (copy_eng == "scalar"
                               or (copy_eng == "alt" and gi % 2))
                    with nc.allow_low_precision(reason="fp16 partial out"):
                        if use_act:
                            nc.scalar.copy(oo[:], ps[:])
                        else:
                            nc.vector.tensor_copy(oo[:], ps[:])
                    nc.sync.dma_start(
                        out=outTs[p][:, dm, jj * 512:(jj + 1) * 512],
                        in_=oo[:])
                return run
            return [group(dm, jj, gi)
                    for gi, (jj, dm) in enumerate(
                        (jj, dm) for jj in jjs for dm in range(NK))]

        # ---------------- attention ----------------
        def make_qk_exp(a, j, ee):
            def qk_exp(p):
                for h in range(2):
                    hb = h * HD
                    sc = pp_qk.tile([128, 2, 512], F32, tag="sc",
                                    name=f"sc{a}{j}{p}{h}")
                    for u in range(2):
                        i = 2 * p + u
                        nc.tensor.matmul(
                            sc[:, u, :],
                            kT[a][hb:hb + HD, i * 128:(i + 1) * 128],
                            q_tiles[a][hb:hb + HD,
                                       j * SQ:(j + 1) * SQ],
                            start=True,
                            stop=True,
                        )
                    e = pool_e.tile([128, 2, 512], BF, tag="e",
                                    name=f"e{a}{j}{p}{h}")
                    nc.scalar.activation(e[:], sc[:], AF.Exp)
                    ee[(p, h)] = e
            return qk_exp

        def av_step(po_, ee_, a_, p):
            """AV for key-tile pair p, both heads, operand-swapped: exp tile
            slices [128k x 128q] are stationary, v_aug [128k x 65] moves —
            65-cycle accumulation steps into [128q, NP, 65] accumulators.
            ONE accumulation group per po bank: start=True zeroes the whole
            PSUM bank, so only the very first matmul of the block carries
            it; every other (qt, i) accumulates into regions the initial
            wipe zeroed."""
            for h in range(2):
                e = ee_.pop((p, h))
                for qt in range(NP):
                    for u in range(2):
                        nc.tensor.matmul(
                            po_[h][:, qt, :],
                            e[:, u, qt * 128:(qt + 1) * 128],
                            v_aug[2 * p + u][:, 2 * a_ + h, :],
                            start=(p == 0 and qt == 0 and u == 0),
                            stop=(p == NIP - 1 and qt == NP - 1
                                  and u == 1),
                            skip_group_check=True,
                        )

        def norm_begin(po_, a_, j_):
            """Reciprocals of the softmax row-sum columns + oq staging tile."""
            with nc.allow_low_precision(reason="bf16 softmax reciprocal"):
                rrs = []
                for h in range(2):
                    rr = pool_rr.tile([128, NP], F32, tag="rr",
                                      name=f"rr{a_}{j_}{h}")
                    nc.vector.reciprocal(rr[:], po_[h][:, :, HD:HD + 1])
                    rrs.append(rr)
                oq = pool_oq.tile([128, NP, 128], BF, tag="oq",
                                  name=f"oq{a_}{j_}")
            return (po_, a_, j_, rrs, oq)

        def norm_part(ns, qts, pe_transpose=False, copy_eng="vector"):
            """Per-partition-scalar normalization multiplies for the given
            query tiles, then [q, d] -> [d, q] via XBAR DMA transposes (or
            PE transposes + DVE copies on the drain path where DMA latency
            would be exposed). qt must run DESCENDING overall: a later
            block's bank-wiping start=True is dep-tracked only against
            region qt0, so qt0's reads must be the LAST norm ops (DVE is
            in-order) for the wipe to serialize behind every read."""
            po_, a_, j_, rrs, oq = ns
            with nc.allow_low_precision(reason="bf16 softmax normalize"):
                for qt in qts:
                    dst = o_tiles[a_][j_][:, qt * 128:(qt + 1) * 128]
                    for h in range(2):
                        nc.vector.tensor_scalar_mul(
                            oq[:, qt, h * HD:(h + 1) * HD],
                            po_[h][:, qt, 0:HD],
                            rrs[h][:, qt:qt + 1])
                    if pe_transpose:
                        pt = pp_fill.tile([128, 128], BF, tag="fl",
                                          name=f"pt{a_}{j_}{qt}")
                        nc.tensor.transpose(pt[:], oq[:, qt, :], ident_t[:])
                        if copy_eng == "scalar":
                            nc.scalar.copy(dst, pt[:])
                        else:
                            nc.vector.tensor_copy(dst, pt[:])
                    else:
                        nc.sync.dma_start_transpose(out=dst,
                                                    in_=oq[:, qt, :])

        def attn_block(a, j, fillers, pre, nxt, carry, pace_off=1,
                       tail_fillers=(), pre_emitted=3):
            """Depth-5 software pipeline over key-tile PAIRS: step p emits
            [QK/exp(p+3), AV(p-2), fillers]; the block's last two AV steps
            and its normalization run in the NEXT block (steps 0-2), so the
            po bank-reuse distance is a full block and the bank-wiping
            start of each AV group never stalls on norm reads. QK/exp runs
            3 steps ahead of issue so the Act exp queue stays fed across
            block-boundary filler stalls; fillers are emitted LAST in each
            step so a stalled filler (psum-ring WAR on a DVE copy) never
            gates the QK convoy. Scores for (2p, 2p+1) share one
            [128,2,512] psum tile per head so exp stays 1024-wide; filler
            matmuls use their own pp_fill pool."""
            po = [pp_av.tile([128, NP, HD + 1], F32, tag="po",
                             name=f"po{a}{j}{h}") for h in range(2)]
            ee = pre
            qk_exp = make_qk_exp(a, j, ee)
            nee = {}
            nqk = make_qk_exp(nxt[0], nxt[1], nee) if nxt else None
            nf, fi = len(fillers), 0
            c_ns = None
            for p in range(NIP):
                if p + 3 < NIP:
                    if p + 3 >= pre_emitted:
                        qk_exp(p + 3)
                elif nqk is not None:
                    nqk(p + 3 - NIP)
                if p < 2:
                    if carry is not None and p == 0:
                        c_po, c_ee, c_a, c_j = carry
                        av_step(c_po, c_ee, c_a, NIP - 2)
                        av_step(c_po, c_ee, c_a, NIP - 1)
                else:
                    av_step(po, ee, a, p - 2)
                if carry is not None and p == 1:
                    # bulk-copy the carry po accumulators to SBUF: ONE fast
                    # DVE read per head frees the po PSUM slots immediately,
                    # so this block's own bank-wiping AV start (p=2) never
                    # waits on the spread-out normalization reads
                    c_po, c_ee, c_a, c_j = carry
                    c_sb = []
                    for h in range(2):
                        sb = pool_ps.tile([128, NP, HD + 1], F32, tag="ps",
                                          name=f"psb{c_a}{c_j}{h}")
                        nc.vector.tensor_copy(sb[:], c_po[h][:])
                        c_sb.append(sb)
                    c_ns = norm_begin(c_sb, c_a, c_j)
                if carry is not None and 2 <= p <= 5:
                    # norm spread ONE query-tile per step: the DVE stream
                    # stays shallow so filler psum-ring copies behind it
                    # release promptly (a norm burst convoys PE via the
                    # in-order DVE queue)
                    norm_part(c_ns, (5 - p,), pe_transpose=True)
                want = -(-(p + pace_off) * nf // NIP)
                while fi < min(want, nf):
                    fillers[fi]()
                    fi += 1
            while fi < nf:
                fillers[fi]()
                fi += 1
            for g in tail_fillers:
                g()
            return nee, (po, ee, a, j)

        q1 = qproj_groups(1)
        q2 = qproj_groups(2)
        q3 = qproj_groups(3)
        k1, k1_dma = kproj_fillers(1)
        k2, k2_dma = kproj_fillers(2)
        k3, k3_dma = kproj_fillers(3)
        op01 = outproj_groups(0, (0, 1, 2, 3))
        op23_0 = outproj_groups(1, (0,))
        op23_1 = outproj_groups(1, (1,))
        def merged_op23(jj, pairs=(2, 3), all_sc=False, ship_each=False):
            """out-proj pairs 2-3 for query chunk jj, staged into 2-dm
            chunk tiles shipped as single DMAs (one HWDGE fixed cost per
            chunk, transfers overlapping the remaining units). ship_each
            ships 1-dm right after each copy — shorter drain for the
            epilogue at the cost of more HWDGE issues."""
            def unit(dm, obc):
                def run():
                    if dm % 2 or all_sc:
                        ps = pp_qk.tile([128, 512], F32, tag="sc",
                                        name=f"pst{jj}{dm}")
                    else:
                        ps = pp_fill.tile([128, 512], F32, tag="fl",
                                         name=f"pst{jj}{dm}")
                    for a in pairs:
                        nc.tensor.matmul(
                            ps[:],
                            wo_t[:, a, dm * 128:(dm + 1) * 128],
                            o_tiles[a][jj][:, :],
                            start=(a == pairs[0]),
                            stop=(a == pairs[-1]),
                        )
                    with nc.allow_low_precision(reason="fp16 partial out"):
                        if dm % 2:
                            nc.scalar.copy(obc[:, dm % 2, :], ps[:])
                        else:
                            nc.vector.tensor_copy(obc[:, dm % 2, :], ps[:])
                return run

            def ship(obc, d0, nd, sl):
                def run():
                    nc.sync.dma_start(
                        out=outTs[1][:, d0:d0 + nd, jj * 512:(jj + 1) * 512],
                        in_=obc[:, sl:sl + nd, :])
                return run

            seq = []
            obc = None
            for dm in range(NK):
                if dm % 2 == 0:
                    obc = pool_ob.tile([128, 2, 512], F16, tag="ob",
                                       name=f"ob{jj}{dm}")
                seq.append(unit(dm, obc))
                if ship_each:
                    seq.append(ship(obc, dm, 1, dm % 2))
                elif dm % 2:
                    seq.append(ship(obc, dm - 1, 2, 0))
            return seq

        op23_2t = merged_op23(2)

        # ---------------- prologue ----------------
        # DMA issue order == consumer order (the modeled DMA transfer device
        # is serial): wk/xk0 then wq/xq0 so the FIRST QK/exp fires ~14 us in;
        # the Act exp chain is the serial long pole, so everything upstream
        # of it is prioritized. k-proj pairs 1-3 re-stream xk later as
        # mid-schedule fillers inside Act-bound blocks.
        xk_dma(0, split=True)
        nc.sync.dma_start(out=wk_t[:, 4:8, 0:256], in_=wkT[:, 4:8, 0:256])
        nc.sync.dma_start(out=bias_t, in_=biasqk[:, :])
        nc.sync.dma_start(out=wq_t, in_=wqT[:, :, :])
        xq_dma(0)
        xk_dma(1)
        kproj_group(0, 0, xk_tiles)
        q0[0]()
        ee0 = {}
        qk00 = make_qk_exp(0, 0, ee0)
        qk00(0)
        qk00(1)
        xk_dma(2)
        kproj_group(0, 1, xk_tiles)
        qk00(2)
        xk_dma(3)
        nc.sync.dma_start(out=wv_t, in_=wvT[:, :, :])
        xv_dma(0)
        kproj_group(0, 2, xk_tiles)
        xv_dma(1)
        kproj_group(0, 3, xk_tiles)
        vt_fill[0]()
        vt_fill[1]()
        xq_dma(1)

        # ------- block (0,0): custom DMA-arrival-aware schedule ----------
        # QK/exp calls pace the Act chain; vt-proj fillers (gated by the
        # serial xv stream) slot between them; AV lags far enough that its
        # v_aug inputs exist. (0,1)'s first five QK calls are pre-emitted
        # here so Act stays fed through the v-proj PE hump at the boundary.
        po00 = [pp_av.tile([128, NP, HD + 1], F32, tag="po",
                           name=f"po00{h}") for h in range(2)]
        nee01 = {}
        qk01 = make_qk_exp(0, 1, nee01)
        qk00(3)
        qk00(4)
        vt_fill[2]()
        vt_fill[3]()
        qk00(5)
        q0[1]()
        qk00(6)
        vt_fill[4]()
        vt_fill[5]()
        qk00(7)
        av_step(po00, ee0, 0, 0)
        qk01(0)
        vt_fill[6]()
        vt_fill[7]()
        av_step(po00, ee0, 0, 1)
        qk01(1)
        vt_fill[8]()
        vt_fill[9]()
        av_step(po00, ee0, 0, 2)
        qk01(2)
        vt_fill[10]()
        vt_fill[11]()
        av_step(po00, ee0, 0, 3)
        qk01(3)
        vt_fill[12]()
        vt_fill[13]()
        av_step(po00, ee0, 0, 4)
        qk01(4)
        av_step(po00, ee0, 0, 5)
        vt_fill[14]()
        vt_fill[15]()
        qk01(5)
        # late-consumer DMAs issued after the xv stream they must not delay
        xq_dma(2)
        xq_dma(3)
        nc.sync.dma_start(out=wk_t[:, :, 256:512], in_=wkT[:, :, 256:512])
        nc.sync.dma_start(out=wo_t, in_=woT[:, :, :])
        nc.sync.dma_start(out=ident_t, in_=identT[:, :])
        k1_dma(0)

        plan = [
            (0, 1, [q0[2], q0[3]], 1, (), 6),
            (0, 2, [k1[0], k1[1], q1[0]], 1, (), 3),
            (0, 3, [k1[2], k1[3], q1[1], lambda: k2_dma(0)], 1, (), 3),
            (1, 0, [q1[2], k2[0], q1[3]], 2, (), 3),
            (1, 1, [k2[1], q2[0]] + op01[0:1], 2, (), 3),
            (1, 2, [k2[2], q2[1]] + op01[1:3], 2, (), 3),
            (1, 3, [k2[3], q2[2]] + op01[3:5] + [lambda: k3_dma(0)],
             2, (), 3),
            (2, 0, [q2[3], k3[0]] + op01[5:8], 2, (), 3),
            (2, 1, [q3[0], k3[1]] + op01[8:11], 2, (), 3),
            (2, 2, [q3[1], k3[2]] + op01[11:14], 2, (), 3),
            (2, 3, [k3[3], q3[2]] + op01[14:17], 2, (), 3),
            (3, 0, [q3[3]] + op01[17:24], 2, (), 3),
            (3, 1, op01[24:32], 2, (), 3),
            (3, 2, op23_0, 2, (), 3),
            # tail fillers (alt copies) hide the final norm chain
            (3, 3, op23_1, 2, op23_2t, 3),
        ]
        pre, carry = nee01, (po00, ee0, 0, 0)
        for bi, (a, j, fillers, off, tails, pe_d) in enumerate(plan):
            nxt = plan[bi + 1][0:2] if bi + 1 < len(plan) else None
            pre, carry = attn_block(a, j, fillers, pre, nxt, carry,
                                    pace_off=off, tail_fillers=tails,
                                    pre_emitted=pe_d)
        # flush: final block's last two AV steps + norm (PE transposes so
        # the drain doesn't pay DMA-transpose issue+sem latency)
        c_po, c_ee, c_a, c_j = carry
        av_step(c_po, c_ee, c_a, NIP - 2)
        av_step(c_po, c_ee, c_a, NIP - 1)
        ns = norm_begin(c_po, c_a, c_j)
        norm_part(ns, (3, 2, 1, 0), pe_transpose=True)
        # epilogue (out-proj pairs 2-3, jj=3): all-sc psum (the score ring
        # is idle at the drain; the fl ring serves the PE transposes)
        for g in merged_op23(3):
            g()

    nc.compile()
    _CACHE["nc"] = nc
    return nc


def _tox(a):
    """[1024|512, N] -> [128, k, N] bf16 (partition-major k-tiling)."""
    r = a.shape[0] // 128
    return np.ascontiguousarray(
        a.reshape(r, 128, a.shape[1]).transpose(1, 0, 2)).astype(NPBF)


def kernel(Q, K, V, Wq, bq, Wk, bk, Wv, bv, Wo, bo):
    Q = np.asarray(Q, np.float32)
    K = np.asarray(K, np.float32)
    V = np.asarray(V, np.float32)
    Wq = np.asarray(Wq, np.float32)
    Wk = np.asarray(Wk, np.float32)
    Wv = np.asarray(Wv, np.float32)
    Wo = np.asarray(Wo, np.float32)
    bq = np.asarray(bq, np.float32)
    bk = np.asarray(bk, np.float32)
    bv = np.asarray(bv, np.float32)
    bo = np.asarray(bo, np.float32)
    scale = 1.0 / 8.0  # 1/sqrt(HD), folded into the q projection

    nc = _build_nc()
    in_maps = []
    for c in range(8):
        b, g = divmod(c, 2)
        gs = slice(g * GS, (g + 1) * GS)
        biasqk = np.empty((128, 8), np.float32)
        for m in range(NP):
            biasqk[:, m] = bq[gs][m * 128:(m + 1) * 128] * scale
            biasqk[:, 4 + m] = bk[gs][m * 128:(m + 1) * 128]
        in_maps.append({
            "xqT": _tox(Q[b].T),
            "xkT": _tox(K[b].T),
            "xvT": _tox(V[b].T),
            "wqT": _tox((Wq[gs] * scale).T),
            "wkT": _tox(Wk[gs].T),
            "wvT": _tox(Wv[gs].T),
            "woT": _tox(Wo[:, gs].T),
            "identT": np.eye(128, dtype=NPBF),
            "biasqk": biasqk,
        })

    host_bias = bo + Wo @ bv  # v bias folded through softmax + out-proj

    def run_and_gather():
        res = run_bass_kernel_spmd(nc, in_maps, list(range(8)))
        out = np.empty((B, S, D), np.float32)
        for b in range(B):
            acc = None
            for c in (2 * b, 2 * b + 1):
                for p in range(2):
                    part = np.asarray(res.results[c][f"outT{p}"])
                    part = part.transpose(1, 0, 2).reshape(D, S)
                    acc = part if acc is None else acc + part
            out[b] = acc.T + host_bias
        return out

    try:
        return run_and_gather()
    except Exception:
        # transient device wedge (e.g. NRT_EXEC_UNIT_UNRECOVERABLE) can
        # surface either in the run or in result materialization: retry once
        return run_and_gather()

